# revision 45
# baseline (speedup 1.0000x reference)
"""NaryTreeLSTM Trainium2 kernel.

Strategy: pure data-parallel over batch (B=32768 -> 4096 rows/core on 8
cores). All on-device compute happens in transposed [h, batch] layout so
matmuls contract over the SBUF partition dim; activations are pre-cast to
fp16 host-side (child o-gate path additionally to fp8e4 for DoubleRow
matmuls: one 256-deep fp8 matmul costs the same as one 128-deep fp16
matmul). The kernel is ScalarE(ACT)-bound: 24 sigmoid/tanh evaluations
per (b,h) element can only run on the activation-LUT engine (~164us/core
of pure streaming at 1 elem/lane/cycle), so everything else is kept under
that line. Gate preactivations accumulate in PSUM (x-path + hsum-path
summed for free; per-partition biases fused into the ACT instruction; the
node i/o biases are pre-added by a rank-2 indicator matmul so sigmoid(i|o)
is one 1024-wide ACT), k-reductions (hsum, c = i*u + sum_k f_k*cc_k) are
short fp16 DVE tree-adds, outputs ship as fp16 and are upcast host-side.
Emission is software-pipelined per 512-column chunk as child_gates(c) |
node_gates(c-1) | child_tail(c) | node_tail(c-1).
"""

import sys

sys.path.insert(0, "/opt/trn_rl_repo")

import numpy as np

B, K, I, H = 32768, 4, 256, 256
NCORES = 8
BLOC = B // NCORES  # 4096 batch rows per core
C = 512  # chunk columns (one PSUM bank of fp32)

_cache = {}


def _build(nchunk):
    """Build the per-core Bass program (identical on all cores)."""
    import concourse.bass as bass  # noqa: F401
    import bass_rust as _bass_rust
    import concourse.tile as tile
    from concourse import bacc, mybir

    f8 = mybir.dt.float8e4
    f16, f32 = mybir.dt.float16, mybir.dt.float32
    AF = mybir.ActivationFunctionType
    DR = mybir.MatmulPerfMode.DoubleRow

    nc = bacc.Bacc("TRN2", target_bir_lowering=False, debug=False, num_devices=NCORES)

    # DRAM I/O. ax packs, per (chunk, itile): [cx_k0|cx_k1|cx_k2|cx_k3|x]
    # blocks of C columns each, rows = 128 contraction indices.
    ax = nc.dram_tensor("ax", [nchunk, 2, 128, 5 * C], f16, kind="ExternalInput").ap()
    # child data again in fp8 with DoubleRow layout [p, it, col]
    cx8 = nc.dram_tensor("cx8", [nchunk, 128, 2, 4 * C], f8, kind="ExternalInput").ap()
    w8o = nc.dram_tensor("w8o", [128, 2, 256], f8, kind="ExternalInput").ap()
    # per-arity forget weights, fp8 DoubleRow layout [p, ct, k*256+h],
    # scaled x64 (the WfT x-path block is also x64 in fp16; the f-gate
    # sigmoid ACT applies scale=1/64)
    wfk8 = nc.dram_tensor("wfk8", [128, 2, 1024], f8, kind="ExternalInput").ap()
    # wcat cols: 0:768 WxiouT (g*256+h), 768:1024 WfT, 1024:2816 UhT
    # ([Ui|Uo|Uu|WfK0..3] at 1024+blk*256+h); rows = contraction index.
    wcat = nc.dram_tensor("wcat", [2, 128, 2816], f16, kind="ExternalInput").ap()
    # bmat cols: 2g+t for g in {i,o,u,f}, t = h-tile
    bmat = nc.dram_tensor("bmat", [128, 8], f32, kind="ExternalInput").ap()
    # node i/o biases in rank-1-matmul lhsT layout [1, 2*ht+g, h]
    b2mat = nc.dram_tensor("b2mat", [1, 4, 128], f16, kind="ExternalInput").ap()
    # all-ones rhs for the rank-1 bias matmuls
    ind2m = nc.dram_tensor("ind2m", [1, C], f16, kind="ExternalInput").ap()
    h_out = nc.dram_tensor("h_out", [256, nchunk * C], f16, kind="ExternalOutput").ap()
    c_out = nc.dram_tensor("c_out", [256, nchunk * C], f16, kind="ExternalOutput").ap()

    with tile.TileContext(nc) as tc:
        import contextlib

        with contextlib.ExitStack() as ctx:
            wpool = ctx.enter_context(tc.tile_pool(name="w", bufs=1))
            apool = ctx.enter_context(tc.tile_pool(name="a", bufs=3))
            gpool = ctx.enter_context(tc.tile_pool(name="g", bufs=1))
            opool = ctx.enter_context(tc.tile_pool(name="o", bufs=2))
            ppool = ctx.enter_context(tc.tile_pool(name="ps", bufs=2, space="PSUM"))

            # weights on the gpsimd DMA queue so the first ax load (sync
            # queue) runs concurrently; x-path weights first so child
            # matmuls can start before the U-path weights arrive.
            b_sb = wpool.tile([128, 8], f32, tag="bias", name="b_sb")
            nc.gpsimd.dma_start(b_sb[:], bmat[:])
            wA, wB = [], []
            for it in range(2):
                a_ = wpool.tile([128, 1024], f16, tag=f"wA{it}", name=f"wA{it}")
                nc.gpsimd.dma_start(a_[:], wcat[it, :, 0:1024])
                wA.append(a_)
            w8 = wpool.tile([128, 2, 256], f8, tag="w8", name="w8")
            nc.gpsimd.dma_start(w8[:], w8o[:])
            wf8 = wpool.tile([128, 2, 1024], f8, tag="wf8", name="wf8")
            nc.gpsimd.dma_start(wf8[:], wfk8[:])
            b2 = wpool.tile([1, 4, 128], f16, tag="b2", name="b2")
            nc.gpsimd.dma_start(b2[:], b2mat[:])
            for it in range(2):
                b_ = wpool.tile([128, 1792], f16, tag=f"wB{it}", name=f"wB{it}")
                nc.gpsimd.dma_start(b_[:], wcat[it, :, 1024:2816])
                wB.append(b_)
            # all-ones rhs for the rank-1 bias matmuls
            ind2 = wpool.tile([1, C], f16, tag="ind2", name="ind2")
            nc.gpsimd.dma_start(ind2[:], ind2m[:])
            # tiny dummy activations so the ACT table load (~2.7us) happens
            # during the initial DMA wait instead of before the first gate
            warm = wpool.tile([1, 8], f32, tag="warm", name="warm")
            nc.vector.memset(warm[:], 0.0)
            nc.scalar.activation(warm[:], warm[:], AF.Sigmoid)

            def wx(it, col):
                return wA[it][:, col : col + 128]

            def wu(it, col):
                return wB[it][:, col - 1024 : col - 1024 + 128]

            def bias(g, ht):
                # g: 0=i, 1=o, 2=u, 3=f
                return b_sb[:, 2 * g + ht : 2 * g + ht + 1]

            def psum4():
                # uniform 4-bank PSUM tiles; bufs=2 -> all 8 banks in flight
                return ppool.tile([128, 4 * C], f32, tag="p4", name="p4")

            def emit_child(c, split=False):
                """Child (leaf) phase for chunk c. Returns live tiles."""
                a_sb = []
                for it in range(2):
                    a = apool.tile([128, 5 * C], f16, tag=f"a{it}", name=f"a{it}")
                    if split:
                        for k in range(5):
                            nc.sync.dma_start(
                                a[:, k * C : (k + 1) * C],
                                ax[c, it, :, k * C : (k + 1) * C],
                            )
                    else:
                        nc.sync.dma_start(a[:], ax[c, it])
                    a_sb.append(a)
                a8 = apool.tile([128, 2, 4 * C], f8, tag="a8", bufs=2, name="a8")
                nc.sync.dma_start(a8[:], cx8[c])
                gates = {}  # (g, ht) -> [128, 4C] fp16 (4 k-blocks)
                for ht in range(2):
                    for g in range(3):
                        gt = gpool.tile(
                            [128, 4 * C], f16, tag=f"cg{g}{ht}", bufs=2,
                            name=f"cg{g}{ht}",
                        )
                        gates[(g, ht)] = gt
                        col = g * 256 + ht * 128
                        fn = AF.Tanh if g == 2 else AF.Sigmoid
                        ps = psum4()
                        if g == 1:
                            # o-gate: fp8 DoubleRow, 4 matmuls instead of 8
                            for k in range(4):
                                nc.tensor.matmul(
                                    ps[:, k * C : (k + 1) * C],
                                    lhsT=w8[:, :, ht * 128 : ht * 128 + 128],
                                    rhs=a8[:, :, k * C : (k + 1) * C],
                                    start=True,
                                    stop=True,
                                    perf_mode=DR,
                                )
                        elif split:
                            # first chunk: k-major + split ACT halves so the
                            # first sigmoid starts after 2 k-block DMAs
                            for k in range(4):
                                for it in range(2):
                                    nc.tensor.matmul(
                                        ps[:, k * C : (k + 1) * C],
                                        lhsT=wx(it, col),
                                        rhs=a_sb[it][:, k * C : (k + 1) * C],
                                        start=(it == 0),
                                        stop=(it == 1),
                                    )
                        else:
                            for it in range(2):  # it-major: same lhsT for 4 MMs
                                for k in range(4):
                                    nc.tensor.matmul(
                                        ps[:, k * C : (k + 1) * C],
                                        lhsT=wx(it, col),
                                        rhs=a_sb[it][:, k * C : (k + 1) * C],
                                        start=(it == 0),
                                        stop=(it == 1),
                                    )
                        if split and g != 1:
                            for kk in range(2):
                                sl = slice(2 * kk * C, 2 * (kk + 1) * C)
                                nc.scalar.activation(
                                    gt[:, sl], ps[:, sl], fn, bias=bias(g, ht)
                                )
                        else:
                            nc.scalar.activation(gt[:], ps[:], fn, bias=bias(g, ht))
                return dict(c=c, a=a_sb, gates=gates)

            def emit_child_tail(st, prev2):
                """cc products, merged tanh([cc(c) | c(c-2)]), hsum tree,
                and (two chunks behind) h = o*tanh(c) + its output DMA."""
                c, gates = st["c"], st["gates"]
                tcm, tct, hs = {}, {}, {}
                hs8 = gpool.tile([128, 2, C], f8, tag="hs8", bufs=2, name="hs8")
                for ht in range(2):
                    # tcm = [cc0..cc3 | c(c-2)]; the c slot was written by
                    # node_tail(c-2) last step (absent for chunks 0/1)
                    if st.get("tcm") and ht in st["tcm"]:
                        tcm[ht] = st["tcm"][ht]
                        full = True
                    else:
                        tcm[ht] = gpool.tile(
                            [128, 5 * C], f16, tag=f"tcm{ht}", bufs=3,
                            name=f"tcm{ht}",
                        )
                        full = False
                    nc.vector.tensor_mul(
                        tcm[ht][:, 0 : 4 * C], gates[(0, ht)][:], gates[(2, ht)][:]
                    )
                    tct[ht] = gpool.tile(
                        [128, 5 * C], f16, tag=f"tct{ht}", bufs=1, name=f"tct{ht}"
                    )
                    if full:
                        nc.scalar.activation(tct[ht][:], tcm[ht][:], AF.Tanh)
                    else:
                        nc.scalar.activation(
                            tct[ht][:, 0 : 4 * C], tcm[ht][:, 0 : 4 * C], AF.Tanh
                        )
                    prod = gpool.tile([128, 4 * C], f16, tag=f"prod{ht}", name="prod")
                    nc.vector.tensor_mul(
                        prod[:], gates[(1, ht)][:], tct[ht][:, 0 : 4 * C]
                    )
                    # hsum = sum_k prod_k, via tree adds
                    t2 = gpool.tile([128, 2 * C], f16, tag=f"t2{ht}", name="t2")
                    nc.vector.tensor_add(t2[:], prod[:, 0 : 2 * C], prod[:, 2 * C : 4 * C])
                    hs[ht] = gpool.tile(
                        [128, C], f16, tag=f"hs{ht}", bufs=3, name=f"hs{ht}"
                    )
                    with nc.allow_low_precision("hsum kept in fp16/fp8 for matmul rhs"):
                        nc.vector.tensor_add(hs[ht][:], t2[:, 0:C], t2[:, C : 2 * C])
                        nc.vector.tensor_copy(hs8[:, ht, :], hs[ht][:])
                    if full:
                        p2 = prev2["c"]
                        h_sb = opool.tile([128, C], f16, tag=f"h{ht}", name="h_sb")
                        nc.vector.tensor_mul(
                            h_sb[:], prev2["io_sb"][ht][:, C : 2 * C],
                            tct[ht][:, 4 * C : 5 * C],
                        )
                        rows = slice(ht * 128, (ht + 1) * 128)
                        cols = slice(p2 * C, (p2 + 1) * C)
                        nc.sync.dma_start(h_out[rows, cols], h_sb[:])
                st["tcm"] = tcm
                st["tct"] = tct
                st["hs"] = hs
                st["hs8"] = hs8

            def emit_node(st):
                c, a_sb, hs = st["c"], st["a"], st["hs"]
                xsl = slice(4 * C, 5 * C)

                st["io_sb"], st["u_sb"] = {}, {}
                for ht in range(2):
                    io_sb = gpool.tile(
                        [128, 2 * C], f16, tag=f"io{ht}", bufs=3, name="io_sb"
                    )
                    # i, o, u gates share one 4-bank tile [i|o|u|unused];
                    # i/o biases pre-added via a rank-2 indicator matmul so
                    # sigmoid(i|o) is one 1024-wide ACT
                    ps_iou = psum4()
                    for it in range(2):
                        for g in range(3):
                            nc.tensor.matmul(
                                ps_iou[:, g * C : (g + 1) * C],
                                lhsT=wx(it, g * 256 + ht * 128),
                                rhs=a_sb[it][:, xsl],
                                start=(it == 0),
                                stop=False,
                            )
                    for gb in range(2):
                        nc.tensor.matmul(
                            ps_iou[:, gb * C : (gb + 1) * C],
                            lhsT=b2[:, 2 * ht + gb, :],
                            rhs=ind2[:],
                            start=False,
                            stop=False,
                        )
                    for ct in range(2):
                        for g in range(3):
                            nc.tensor.matmul(
                                ps_iou[:, g * C : (g + 1) * C],
                                lhsT=wu(ct, 1024 + g * 256 + ht * 128),
                                rhs=hs[ct][:],
                                start=False,
                                stop=(ct == 1),
                            )
                    nc.scalar.activation(io_sb[:], ps_iou[:, 0 : 2 * C], AF.Sigmoid)
                    u_sb = gpool.tile([128, C], f16, tag=f"u{ht}", bufs=2, name="u_sb")
                    nc.scalar.activation(
                        u_sb[:], ps_iou[:, 2 * C : 3 * C], AF.Tanh, bias=bias(2, ht)
                    )
                    st["u_sb"][ht] = u_sb
                    # f gates for all 4 children in one 4-bank tile (the
                    # fx = Wf@x term is re-accumulated per k on the PE; the
                    # redundant matmuls are cheaper than any add elsewhere)
                    psf = psum4()
                    for it in range(2):
                        for k in range(4):
                            nc.tensor.matmul(
                                psf[:, k * C : (k + 1) * C],
                                lhsT=wx(it, 768 + ht * 128),
                                rhs=a_sb[it][:, xsl],
                                start=(it == 0),
                                stop=False,
                            )
                    for k in range(4):
                        nc.tensor.matmul(
                            psf[:, k * C : (k + 1) * C],
                            lhsT=wf8[
                                :, :, k * 256 + ht * 128 : k * 256 + ht * 128 + 128
                            ],
                            rhs=st["hs8"][:],
                            start=False,
                            stop=True,
                            perf_mode=DR,
                        )
                    f_sb = gpool.tile([128, 4 * C], f16, tag=f"f{ht}", name="f_sb")
                    nc.scalar.activation(
                        f_sb[:], psf[:], AF.Sigmoid, bias=bias(3, ht), scale=1.0 / 64.0
                    )
                    st["io_sb"][ht] = io_sb
                    st.setdefault("f_sb", {})[ht] = f_sb

            def emit_node_tail(st, pending):
                """c(p) = i*u + sum_k f_k*cc_k, written into the NEXT-NEXT
                chunk's tcm tile (tanh'd there by the merged ACT); c output
                DMA'd now. h(p) happens in child_tail(p+2)."""
                c = st["c"]
                lp = nc.allow_low_precision
                for ht in range(2):
                    f_sb, io_sb = st["f_sb"][ht], st["io_sb"][ht]
                    tcm_p = st["tcm"][ht]
                    prod4 = gpool.tile([128, 4 * C], f16, tag=f"p4d{ht}", name="prod4")
                    nc.vector.tensor_mul(prod4[:], f_sb[:], tcm_p[:, 0 : 4 * C])
                    piu = gpool.tile([128, C], f16, tag=f"piu{ht}", name="piu")
                    nc.vector.tensor_mul(piu[:], io_sb[:, 0:C], st["u_sb"][ht][:])
                    t1 = gpool.tile([128, 2 * C], f16, tag=f"t1{ht}", name="t1")
                    tcm_n = gpool.tile(
                        [128, 5 * C], f16, tag=f"tcm{ht}", bufs=3, name=f"tcm{ht}"
                    )
                    with lp("fp16 tree-add of fp16 products"):
                        nc.vector.tensor_add(
                            t1[:], prod4[:, 0 : 2 * C], prod4[:, 2 * C : 4 * C]
                        )
                        t2 = gpool.tile([128, C], f16, tag=f"t2n{ht}", name="t2n")
                        nc.vector.tensor_add(t2[:], t1[:, 0:C], t1[:, C : 2 * C])
                        nc.vector.tensor_add(
                            tcm_n[:, 4 * C : 5 * C], t2[:], piu[:]
                        )
                    rows = slice(ht * 128, (ht + 1) * 128)
                    cols = slice(c * C, (c + 1) * C)
                    nc.sync.dma_start(c_out[rows, cols], tcm_n[:, 4 * C : 5 * C])
                    pending[ht] = tcm_n

            # Software pipeline per step c:
            #   child_gates(c) | node_gates(c-1) | child_tail(c) (merged
            #   tanh + h(c-2)) | node_tail(c-1) (c into tcm(c+1))
            prev, prev2 = None, None
            pending = {}
            for c in range(nchunk):
                cur = emit_child(c, split=(c == 0))
                if pending:
                    cur["tcm"] = dict(pending)
                    pending.clear()
                if prev is not None:
                    emit_node(prev)
                emit_child_tail(cur, prev2)
                if prev is not None:
                    emit_node_tail(prev, pending)
                prev2, prev = prev, cur
            emit_node(prev)
            emit_node_tail(prev, pending2 := {})
            # epilogue: tanh + h for the last two chunks (their c landed in
            # tcm tiles that no later child_tail will tanh)
            for st, pend in ((prev2, pending), (prev, pending2)):
                p = st["c"]
                for ht in range(2):
                    tcn = pend[ht]
                    tc_sb = gpool.tile([128, C], f16, tag=f"tc{ht}", name="tc_sb")
                    nc.scalar.activation(
                        tc_sb[:], tcn[:, 4 * C : 5 * C], AF.Tanh
                    )
                    h_sb = opool.tile([128, C], f16, tag=f"h{ht}", name="h_sb")
                    nc.vector.tensor_mul(
                        h_sb[:], st["io_sb"][ht][:, C : 2 * C], tc_sb[:]
                    )
                    rows = slice(ht * 128, (ht + 1) * 128)
                    cols = slice(p * C, (p + 1) * C)
                    nc.sync.dma_start(h_out[rows, cols], h_sb[:])

    nc.compile()
    return nc


def _prep_shared(Wi, bi, Wf, bf, Wo, bo, Wu, bu, Ui, Uo, Uu, WfK):
    """Weight/bias packing shared by all cores."""
    import ml_dtypes

    WxiouT = np.concatenate([Wi, Wo, Wu], axis=0).T  # [256, 768]
    WfT = 64.0 * np.asarray(Wf, np.float32).T  # [256, 256], x64 for fp8 WfK path
    UhT = np.concatenate([Ui, Uo, Uu, WfK[0], WfK[1], WfK[2], WfK[3]], axis=0).T
    wcat = np.concatenate([WxiouT, WfT, UhT], axis=1).astype(np.float16)  # [256, 2816]
    wcat = np.ascontiguousarray(wcat.reshape(2, 128, 2816))

    bmat = np.empty((128, 8), np.float32)
    for g, b in enumerate([bi, bo, bu, bf]):
        b = np.asarray(b, np.float32)
        bmat[:, 2 * g] = b[:128]
        bmat[:, 2 * g + 1] = b[128:]

    b2mat = np.empty((1, 4, 128), np.float16)
    for ht in range(2):
        b2mat[0, 2 * ht + 0] = np.asarray(bi, np.float32)[ht * 128 : (ht + 1) * 128]
        b2mat[0, 2 * ht + 1] = np.asarray(bo, np.float32)[ht * 128 : (ht + 1) * 128]

    ind2m = np.ones((1, C), np.float16)

    # fp8 o-gate weights, DoubleRow layout [p, it, hcol]
    w8o = np.ascontiguousarray(
        np.asarray(Wo, np.float32).T.reshape(2, 128, 256).transpose(1, 0, 2)
    ).astype(ml_dtypes.float8_e4m3)
    # fp8 forget weights x64, DoubleRow layout [p, ct, k*256+h]
    wfkT = np.concatenate(
        [64.0 * np.asarray(WfK[k], np.float32).T for k in range(4)], axis=1
    )  # [256, 1024]
    wfk8 = np.ascontiguousarray(
        wfkT.reshape(2, 128, 1024).transpose(1, 0, 2)
    ).astype(ml_dtypes.float8_e4m3)
    return wcat, bmat, b2mat, ind2m, w8o, wfk8


def _prep_core(x, child_x, m, nchunk):
    """Pack per-core activations: [nchunk, 2, 128, 5C] fp16 + fp8 child."""
    import ml_dtypes

    bloc = nchunk * C
    sl = slice(m * bloc, (m + 1) * bloc)
    cxt = np.asarray(child_x[sl], np.float16).transpose(2, 1, 0)  # [256, 4, bloc]
    xt = np.asarray(x[sl], np.float16).T[:, None, :]  # [256, 1, bloc]
    full = np.concatenate([cxt, xt], axis=1)  # [256, 5, bloc]
    # [it, p, j, chunk, cb] -> [chunk, it, p, j, cb]
    full = full.reshape(2, 128, 5, nchunk, C).transpose(3, 0, 1, 2, 4)
    ax = np.ascontiguousarray(full).reshape(nchunk, 2, 128, 5 * C)
    # fp8 child blocks only, DoubleRow layout [chunk, p, it, 4C]
    cx8 = np.ascontiguousarray(
        ax[:, :, :, 0 : 4 * C].transpose(0, 2, 1, 3)
    ).astype(ml_dtypes.float8_e4m3)
    return ax, cx8


def _run(inputs, nchunk, trace=False):
    from concourse.bass_utils import run_bass_kernel_spmd

    key = ("nc", nchunk)
    if key not in _cache:
        _cache[key] = _build(nchunk)
    nc = _cache[key]

    wcat, bmat, b2mat, ind2m, w8o, wfk8 = _prep_shared(
        inputs["Wi"], inputs["bi"], inputs["Wf"], inputs["bf"],
        inputs["Wo"], inputs["bo"], inputs["Wu"], inputs["bu"],
        inputs["Ui"], inputs["Uo"], inputs["Uu"], inputs["WfK"],
    )
    in_maps = []
    for m in range(NCORES):
        ax, cx8 = _prep_core(inputs["x"], inputs["child_x"], m, nchunk)
        in_maps.append(
            {"ax": ax, "cx8": cx8, "wcat": wcat, "bmat": bmat,
             "b2mat": b2mat, "ind2m": ind2m, "w8o": w8o, "wfk8": wfk8}
        )

    res = run_bass_kernel_spmd(
        nc, in_maps, core_ids=list(range(NCORES)), trace=trace
    )
    bloc = nchunk * C
    h = np.empty((NCORES * bloc, 256), np.float32)
    c = np.empty((NCORES * bloc, 256), np.float32)
    for m, r in enumerate(res.results):
        h[m * bloc : (m + 1) * bloc] = r["h_out"].T.astype(np.float32)
        c[m * bloc : (m + 1) * bloc] = r["c_out"].T.astype(np.float32)
    return (h, c), res


def kernel(**inputs):
    (h, c), _ = _run(inputs, BLOC // C)
    return h, c


# revision 47
# speedup vs baseline: 1.0054x; 1.0054x over previous
"""NaryTreeLSTM Trainium2 kernel.

Strategy: pure data-parallel over batch (B=32768 -> 4096 rows/core on 8
cores). All on-device compute happens in transposed [h, batch] layout so
matmuls contract over the SBUF partition dim; activations are pre-cast to
fp16 host-side (child o-gate path additionally to fp8e4 for DoubleRow
matmuls: one 256-deep fp8 matmul costs the same as one 128-deep fp16
matmul). The kernel is ScalarE(ACT)-bound: 24 sigmoid/tanh evaluations
per (b,h) element can only run on the activation-LUT engine (~164us/core
of pure streaming at 1 elem/lane/cycle), so everything else is kept under
that line. Gate preactivations accumulate in PSUM (x-path + hsum-path
summed for free; per-partition biases fused into the ACT instruction; the
node i/o biases are pre-added by a rank-2 indicator matmul so sigmoid(i|o)
is one 1024-wide ACT), k-reductions (hsum, c = i*u + sum_k f_k*cc_k) are
short fp16 DVE tree-adds, outputs ship as fp16 and are upcast host-side.
Emission is software-pipelined per 512-column chunk as child_gates(c) |
node_gates(c-1) | child_tail(c) | node_tail(c-1).
"""

import sys

sys.path.insert(0, "/opt/trn_rl_repo")

import numpy as np

B, K, I, H = 32768, 4, 256, 256
NCORES = 8
BLOC = B // NCORES  # 4096 batch rows per core
C = 512  # chunk columns (one PSUM bank of fp32)

_cache = {}


def _build(nchunk):
    """Build the per-core Bass program (identical on all cores)."""
    import concourse.bass as bass  # noqa: F401
    import bass_rust as _bass_rust
    import concourse.tile as tile
    from concourse import bacc, mybir

    f8 = mybir.dt.float8e4
    f16, f32 = mybir.dt.float16, mybir.dt.float32
    AF = mybir.ActivationFunctionType
    DR = mybir.MatmulPerfMode.DoubleRow

    nc = bacc.Bacc("TRN2", target_bir_lowering=False, debug=False, num_devices=NCORES)

    # DRAM I/O. ax packs, per (chunk, itile): [cx_k0|cx_k1|cx_k2|cx_k3|x]
    # blocks of C columns each, rows = 128 contraction indices.
    ax = nc.dram_tensor("ax", [nchunk, 2, 128, 5 * C], f16, kind="ExternalInput").ap()
    # child data again in fp8 with DoubleRow layout [p, it, col]
    cx8 = nc.dram_tensor("cx8", [nchunk, 128, 2, 4 * C], f8, kind="ExternalInput").ap()
    w8o = nc.dram_tensor("w8o", [128, 2, 256], f8, kind="ExternalInput").ap()
    # per-arity forget weights, fp8 DoubleRow layout [p, ct, k*256+h],
    # scaled x64 (the WfT x-path block is also x64 in fp16; the f-gate
    # sigmoid ACT applies scale=1/64)
    wfk8 = nc.dram_tensor("wfk8", [128, 2, 1024], f8, kind="ExternalInput").ap()
    # wcat cols: 0:768 WxiouT (g*256+h), 768:1024 WfT, 1024:2816 UhT
    # ([Ui|Uo|Uu|WfK0..3] at 1024+blk*256+h); rows = contraction index.
    wcat = nc.dram_tensor("wcat", [2, 128, 2816], f16, kind="ExternalInput").ap()
    # bmat cols: 2g+t for g in {i,o,u,f}, t = h-tile
    bmat = nc.dram_tensor("bmat", [128, 8], f32, kind="ExternalInput").ap()
    # node i/o biases in rank-1-matmul lhsT layout [1, 2*ht+g, h]
    b2mat = nc.dram_tensor("b2mat", [1, 4, 128], f16, kind="ExternalInput").ap()
    # all-ones rhs for the rank-1 bias matmuls
    ind2m = nc.dram_tensor("ind2m", [1, C], f16, kind="ExternalInput").ap()
    h_out = nc.dram_tensor("h_out", [256, nchunk * C], f16, kind="ExternalOutput").ap()
    c_out = nc.dram_tensor("c_out", [256, nchunk * C], f16, kind="ExternalOutput").ap()

    with tile.TileContext(nc) as tc:
        import contextlib

        with contextlib.ExitStack() as ctx:
            wpool = ctx.enter_context(tc.tile_pool(name="w", bufs=1))
            apool = ctx.enter_context(tc.tile_pool(name="a", bufs=3))
            gpool = ctx.enter_context(tc.tile_pool(name="g", bufs=1))
            opool = ctx.enter_context(tc.tile_pool(name="o", bufs=2))
            ppool = ctx.enter_context(tc.tile_pool(name="ps", bufs=2, space="PSUM"))

            # weights on the gpsimd DMA queue so the first ax load (sync
            # queue) runs concurrently; x-path weights first so child
            # matmuls can start before the U-path weights arrive.
            b_sb = wpool.tile([128, 8], f32, tag="bias", name="b_sb")
            nc.gpsimd.dma_start(b_sb[:], bmat[:])
            wA, wB = [], []
            for it in range(2):
                a_ = wpool.tile([128, 1024], f16, tag=f"wA{it}", name=f"wA{it}")
                nc.gpsimd.dma_start(a_[:], wcat[it, :, 0:1024])
                wA.append(a_)
            w8 = wpool.tile([128, 2, 256], f8, tag="w8", name="w8")
            nc.gpsimd.dma_start(w8[:], w8o[:])
            wf8 = wpool.tile([128, 2, 1024], f8, tag="wf8", name="wf8")
            nc.gpsimd.dma_start(wf8[:], wfk8[:])
            b2 = wpool.tile([1, 4, 128], f16, tag="b2", name="b2")
            nc.gpsimd.dma_start(b2[:], b2mat[:])
            for it in range(2):
                b_ = wpool.tile([128, 1792], f16, tag=f"wB{it}", name=f"wB{it}")
                wB.append(b_)

            def load_wB():
                # U-path weights aren't needed until the first node phase
                # (step 1) - issuing them after chunk 0's activation DMAs
                # keeps the gpsimd queue free for the startup-critical loads
                for it in range(2):
                    nc.gpsimd.dma_start(wB[it][:], wcat[it, :, 1024:2816])
            # all-ones rhs for the rank-1 bias matmuls
            ind2 = wpool.tile([1, C], f16, tag="ind2", name="ind2")
            nc.gpsimd.dma_start(ind2[:], ind2m[:])
            # tiny dummy activations so the ACT table load (~2.7us) happens
            # during the initial DMA wait instead of before the first gate
            warm = wpool.tile([1, 8], f32, tag="warm", name="warm")
            nc.vector.memset(warm[:], 0.0)
            nc.scalar.activation(warm[:], warm[:], AF.Sigmoid)

            def wx(it, col):
                return wA[it][:, col : col + 128]

            def wu(it, col):
                return wB[it][:, col - 1024 : col - 1024 + 128]

            def bias(g, ht):
                # g: 0=i, 1=o, 2=u, 3=f
                return b_sb[:, 2 * g + ht : 2 * g + ht + 1]

            def psum4():
                # uniform 4-bank PSUM tiles; bufs=2 -> all 8 banks in flight
                return ppool.tile([128, 4 * C], f32, tag="p4", name="p4")

            def emit_child(c, split=False):
                """Child (leaf) phase for chunk c. Returns live tiles."""
                a_sb = []
                for it in range(2):
                    a = apool.tile([128, 5 * C], f16, tag=f"a{it}", name=f"a{it}")
                    if split:
                        eng = nc.gpsimd if it == 1 else nc.sync
                        for k in range(5):
                            eng.dma_start(
                                a[:, k * C : (k + 1) * C],
                                ax[c, it, :, k * C : (k + 1) * C],
                            )
                    else:
                        nc.sync.dma_start(a[:], ax[c, it])
                    a_sb.append(a)
                a8 = apool.tile([128, 2, 4 * C], f8, tag="a8", bufs=2, name="a8")
                nc.sync.dma_start(a8[:], cx8[c])
                gates = {}  # (g, ht) -> [128, 4C] fp16 (4 k-blocks)
                for ht in range(2):
                    for g in range(3):
                        gt = gpool.tile(
                            [128, 4 * C], f16, tag=f"cg{g}{ht}", bufs=2,
                            name=f"cg{g}{ht}",
                        )
                        gates[(g, ht)] = gt
                        col = g * 256 + ht * 128
                        fn = AF.Tanh if g == 2 else AF.Sigmoid
                        ps = psum4()
                        if g == 1:
                            # o-gate: fp8 DoubleRow, 4 matmuls instead of 8
                            for k in range(4):
                                nc.tensor.matmul(
                                    ps[:, k * C : (k + 1) * C],
                                    lhsT=w8[:, :, ht * 128 : ht * 128 + 128],
                                    rhs=a8[:, :, k * C : (k + 1) * C],
                                    start=True,
                                    stop=True,
                                    perf_mode=DR,
                                )
                        else:
                            for it in range(2):  # it-major: same lhsT for 4 MMs
                                for k in range(4):
                                    nc.tensor.matmul(
                                        ps[:, k * C : (k + 1) * C],
                                        lhsT=wx(it, col),
                                        rhs=a_sb[it][:, k * C : (k + 1) * C],
                                        start=(it == 0),
                                        stop=(it == 1),
                                    )
                        nc.scalar.activation(gt[:], ps[:], fn, bias=bias(g, ht))
                return dict(c=c, a=a_sb, gates=gates)

            def emit_child_tail(st, prev2):
                """cc products, merged tanh([cc(c) | c(c-2)]), hsum tree,
                and (two chunks behind) h = o*tanh(c) + its output DMA."""
                c, gates = st["c"], st["gates"]
                tcm, tct, hs = {}, {}, {}
                hs8 = gpool.tile([128, 2, C], f8, tag="hs8", bufs=2, name="hs8")
                for ht in range(2):
                    # tcm = [cc0..cc3 | c(c-2)]; the c slot was written by
                    # node_tail(c-2) last step (absent for chunks 0/1)
                    if st.get("tcm") and ht in st["tcm"]:
                        tcm[ht] = st["tcm"][ht]
                        full = True
                    else:
                        tcm[ht] = gpool.tile(
                            [128, 5 * C], f16, tag=f"tcm{ht}", bufs=3,
                            name=f"tcm{ht}",
                        )
                        full = False
                    nc.vector.tensor_mul(
                        tcm[ht][:, 0 : 4 * C], gates[(0, ht)][:], gates[(2, ht)][:]
                    )
                    tct[ht] = gpool.tile(
                        [128, 5 * C], f16, tag=f"tct{ht}", bufs=1, name=f"tct{ht}"
                    )
                    if full:
                        nc.scalar.activation(tct[ht][:], tcm[ht][:], AF.Tanh)
                    else:
                        nc.scalar.activation(
                            tct[ht][:, 0 : 4 * C], tcm[ht][:, 0 : 4 * C], AF.Tanh
                        )
                    prod = gpool.tile([128, 4 * C], f16, tag=f"prod{ht}", name="prod")
                    nc.vector.tensor_mul(
                        prod[:], gates[(1, ht)][:], tct[ht][:, 0 : 4 * C]
                    )
                    # hsum = sum_k prod_k, via tree adds
                    t2 = gpool.tile([128, 2 * C], f16, tag=f"t2{ht}", name="t2")
                    nc.vector.tensor_add(t2[:], prod[:, 0 : 2 * C], prod[:, 2 * C : 4 * C])
                    hs[ht] = gpool.tile(
                        [128, C], f16, tag=f"hs{ht}", bufs=3, name=f"hs{ht}"
                    )
                    with nc.allow_low_precision("hsum kept in fp16/fp8 for matmul rhs"):
                        nc.vector.tensor_add(hs[ht][:], t2[:, 0:C], t2[:, C : 2 * C])
                        nc.vector.tensor_copy(hs8[:, ht, :], hs[ht][:])
                    if full:
                        p2 = prev2["c"]
                        h_sb = opool.tile([128, C], f16, tag=f"h{ht}", name="h_sb")
                        nc.vector.tensor_mul(
                            h_sb[:], prev2["io_sb"][ht][:, C : 2 * C],
                            tct[ht][:, 4 * C : 5 * C],
                        )
                        rows = slice(ht * 128, (ht + 1) * 128)
                        cols = slice(p2 * C, (p2 + 1) * C)
                        nc.sync.dma_start(h_out[rows, cols], h_sb[:])
                st["tcm"] = tcm
                st["tct"] = tct
                st["hs"] = hs
                st["hs8"] = hs8

            def emit_node(st):
                c, a_sb, hs = st["c"], st["a"], st["hs"]
                xsl = slice(4 * C, 5 * C)

                st["io_sb"], st["u_sb"] = {}, {}
                for ht in range(2):
                    io_sb = gpool.tile(
                        [128, 2 * C], f16, tag=f"io{ht}", bufs=3, name="io_sb"
                    )
                    # i, o, u gates share one 4-bank tile [i|o|u|unused];
                    # i/o biases pre-added via a rank-2 indicator matmul so
                    # sigmoid(i|o) is one 1024-wide ACT
                    ps_iou = psum4()
                    for it in range(2):
                        for g in range(3):
                            nc.tensor.matmul(
                                ps_iou[:, g * C : (g + 1) * C],
                                lhsT=wx(it, g * 256 + ht * 128),
                                rhs=a_sb[it][:, xsl],
                                start=(it == 0),
                                stop=False,
                            )
                    for gb in range(2):
                        nc.tensor.matmul(
                            ps_iou[:, gb * C : (gb + 1) * C],
                            lhsT=b2[:, 2 * ht + gb, :],
                            rhs=ind2[:],
                            start=False,
                            stop=False,
                        )
                    for ct in range(2):
                        for g in range(3):
                            nc.tensor.matmul(
                                ps_iou[:, g * C : (g + 1) * C],
                                lhsT=wu(ct, 1024 + g * 256 + ht * 128),
                                rhs=hs[ct][:],
                                start=False,
                                stop=(ct == 1),
                            )
                    nc.scalar.activation(io_sb[:], ps_iou[:, 0 : 2 * C], AF.Sigmoid)
                    u_sb = gpool.tile([128, C], f16, tag=f"u{ht}", bufs=2, name="u_sb")
                    nc.scalar.activation(
                        u_sb[:], ps_iou[:, 2 * C : 3 * C], AF.Tanh, bias=bias(2, ht)
                    )
                    st["u_sb"][ht] = u_sb
                    # f gates for all 4 children in one 4-bank tile (the
                    # fx = Wf@x term is re-accumulated per k on the PE; the
                    # redundant matmuls are cheaper than any add elsewhere)
                    psf = psum4()
                    for it in range(2):
                        for k in range(4):
                            nc.tensor.matmul(
                                psf[:, k * C : (k + 1) * C],
                                lhsT=wx(it, 768 + ht * 128),
                                rhs=a_sb[it][:, xsl],
                                start=(it == 0),
                                stop=False,
                            )
                    for k in range(4):
                        nc.tensor.matmul(
                            psf[:, k * C : (k + 1) * C],
                            lhsT=wf8[
                                :, :, k * 256 + ht * 128 : k * 256 + ht * 128 + 128
                            ],
                            rhs=st["hs8"][:],
                            start=False,
                            stop=True,
                            perf_mode=DR,
                        )
                    f_sb = gpool.tile([128, 4 * C], f16, tag=f"f{ht}", name="f_sb")
                    nc.scalar.activation(
                        f_sb[:], psf[:], AF.Sigmoid, bias=bias(3, ht), scale=1.0 / 64.0
                    )
                    st["io_sb"][ht] = io_sb
                    st.setdefault("f_sb", {})[ht] = f_sb

            def emit_node_tail(st, pending):
                """c(p) = i*u + sum_k f_k*cc_k, written into the NEXT-NEXT
                chunk's tcm tile (tanh'd there by the merged ACT); c output
                DMA'd now. h(p) happens in child_tail(p+2)."""
                c = st["c"]
                lp = nc.allow_low_precision
                for ht in range(2):
                    f_sb, io_sb = st["f_sb"][ht], st["io_sb"][ht]
                    tcm_p = st["tcm"][ht]
                    prod4 = gpool.tile([128, 4 * C], f16, tag=f"p4d{ht}", name="prod4")
                    nc.vector.tensor_mul(prod4[:], f_sb[:], tcm_p[:, 0 : 4 * C])
                    piu = gpool.tile([128, C], f16, tag=f"piu{ht}", name="piu")
                    nc.vector.tensor_mul(piu[:], io_sb[:, 0:C], st["u_sb"][ht][:])
                    t1 = gpool.tile([128, 2 * C], f16, tag=f"t1{ht}", name="t1")
                    tcm_n = gpool.tile(
                        [128, 5 * C], f16, tag=f"tcm{ht}", bufs=3, name=f"tcm{ht}"
                    )
                    with lp("fp16 tree-add of fp16 products"):
                        nc.vector.tensor_add(
                            t1[:], prod4[:, 0 : 2 * C], prod4[:, 2 * C : 4 * C]
                        )
                        t2 = gpool.tile([128, C], f16, tag=f"t2n{ht}", name="t2n")
                        nc.vector.tensor_add(t2[:], t1[:, 0:C], t1[:, C : 2 * C])
                        nc.vector.tensor_add(
                            tcm_n[:, 4 * C : 5 * C], t2[:], piu[:]
                        )
                    rows = slice(ht * 128, (ht + 1) * 128)
                    cols = slice(c * C, (c + 1) * C)
                    nc.sync.dma_start(c_out[rows, cols], tcm_n[:, 4 * C : 5 * C])
                    pending[ht] = tcm_n

            # Software pipeline per step c:
            #   child_gates(c) | node_gates(c-1) | child_tail(c) (merged
            #   tanh + h(c-2)) | node_tail(c-1) (c into tcm(c+1))
            prev, prev2 = None, None
            pending = {}
            for c in range(nchunk):
                cur = emit_child(c, split=(c == 0))
                if c == 0:
                    load_wB()
                if pending:
                    cur["tcm"] = dict(pending)
                    pending.clear()
                if prev is not None:
                    emit_node(prev)
                emit_child_tail(cur, prev2)
                if prev is not None:
                    emit_node_tail(prev, pending)
                prev2, prev = prev, cur
            emit_node(prev)
            emit_node_tail(prev, pending2 := {})
            # epilogue: tanh + h for the last two chunks (their c landed in
            # tcm tiles that no later child_tail will tanh)
            for st, pend in ((prev2, pending), (prev, pending2)):
                p = st["c"]
                for ht in range(2):
                    tcn = pend[ht]
                    tc_sb = gpool.tile([128, C], f16, tag=f"tc{ht}", name="tc_sb")
                    nc.scalar.activation(
                        tc_sb[:], tcn[:, 4 * C : 5 * C], AF.Tanh
                    )
                    h_sb = opool.tile([128, C], f16, tag=f"h{ht}", name="h_sb")
                    nc.vector.tensor_mul(
                        h_sb[:], st["io_sb"][ht][:, C : 2 * C], tc_sb[:]
                    )
                    rows = slice(ht * 128, (ht + 1) * 128)
                    cols = slice(p * C, (p + 1) * C)
                    nc.sync.dma_start(h_out[rows, cols], h_sb[:])

    nc.compile()
    return nc


def _prep_shared(Wi, bi, Wf, bf, Wo, bo, Wu, bu, Ui, Uo, Uu, WfK):
    """Weight/bias packing shared by all cores."""
    import ml_dtypes

    WxiouT = np.concatenate([Wi, Wo, Wu], axis=0).T  # [256, 768]
    WfT = 64.0 * np.asarray(Wf, np.float32).T  # [256, 256], x64 for fp8 WfK path
    UhT = np.concatenate([Ui, Uo, Uu, WfK[0], WfK[1], WfK[2], WfK[3]], axis=0).T
    wcat = np.concatenate([WxiouT, WfT, UhT], axis=1).astype(np.float16)  # [256, 2816]
    wcat = np.ascontiguousarray(wcat.reshape(2, 128, 2816))

    bmat = np.empty((128, 8), np.float32)
    for g, b in enumerate([bi, bo, bu, bf]):
        b = np.asarray(b, np.float32)
        bmat[:, 2 * g] = b[:128]
        bmat[:, 2 * g + 1] = b[128:]

    b2mat = np.empty((1, 4, 128), np.float16)
    for ht in range(2):
        b2mat[0, 2 * ht + 0] = np.asarray(bi, np.float32)[ht * 128 : (ht + 1) * 128]
        b2mat[0, 2 * ht + 1] = np.asarray(bo, np.float32)[ht * 128 : (ht + 1) * 128]

    ind2m = np.ones((1, C), np.float16)

    # fp8 o-gate weights, DoubleRow layout [p, it, hcol]
    w8o = np.ascontiguousarray(
        np.asarray(Wo, np.float32).T.reshape(2, 128, 256).transpose(1, 0, 2)
    ).astype(ml_dtypes.float8_e4m3)
    # fp8 forget weights x64, DoubleRow layout [p, ct, k*256+h]
    wfkT = np.concatenate(
        [64.0 * np.asarray(WfK[k], np.float32).T for k in range(4)], axis=1
    )  # [256, 1024]
    wfk8 = np.ascontiguousarray(
        wfkT.reshape(2, 128, 1024).transpose(1, 0, 2)
    ).astype(ml_dtypes.float8_e4m3)
    return wcat, bmat, b2mat, ind2m, w8o, wfk8


def _prep_core(x, child_x, m, nchunk):
    """Pack per-core activations: [nchunk, 2, 128, 5C] fp16 + fp8 child."""
    import ml_dtypes

    bloc = nchunk * C
    sl = slice(m * bloc, (m + 1) * bloc)
    cxt = np.asarray(child_x[sl], np.float16).transpose(2, 1, 0)  # [256, 4, bloc]
    xt = np.asarray(x[sl], np.float16).T[:, None, :]  # [256, 1, bloc]
    full = np.concatenate([cxt, xt], axis=1)  # [256, 5, bloc]
    # [it, p, j, chunk, cb] -> [chunk, it, p, j, cb]
    full = full.reshape(2, 128, 5, nchunk, C).transpose(3, 0, 1, 2, 4)
    ax = np.ascontiguousarray(full).reshape(nchunk, 2, 128, 5 * C)
    # fp8 child blocks only, DoubleRow layout [chunk, p, it, 4C]
    cx8 = np.ascontiguousarray(
        ax[:, :, :, 0 : 4 * C].transpose(0, 2, 1, 3)
    ).astype(ml_dtypes.float8_e4m3)
    return ax, cx8


def _run(inputs, nchunk, trace=False):
    from concourse.bass_utils import run_bass_kernel_spmd

    key = ("nc", nchunk)
    if key not in _cache:
        _cache[key] = _build(nchunk)
    nc = _cache[key]

    wcat, bmat, b2mat, ind2m, w8o, wfk8 = _prep_shared(
        inputs["Wi"], inputs["bi"], inputs["Wf"], inputs["bf"],
        inputs["Wo"], inputs["bo"], inputs["Wu"], inputs["bu"],
        inputs["Ui"], inputs["Uo"], inputs["Uu"], inputs["WfK"],
    )
    in_maps = []
    for m in range(NCORES):
        ax, cx8 = _prep_core(inputs["x"], inputs["child_x"], m, nchunk)
        in_maps.append(
            {"ax": ax, "cx8": cx8, "wcat": wcat, "bmat": bmat,
             "b2mat": b2mat, "ind2m": ind2m, "w8o": w8o, "wfk8": wfk8}
        )

    res = run_bass_kernel_spmd(
        nc, in_maps, core_ids=list(range(NCORES)), trace=trace
    )
    bloc = nchunk * C
    h = np.empty((NCORES * bloc, 256), np.float32)
    c = np.empty((NCORES * bloc, 256), np.float32)
    for m, r in enumerate(res.results):
        h[m * bloc : (m + 1) * bloc] = r["h_out"].T.astype(np.float32)
        c[m * bloc : (m + 1) * bloc] = r["c_out"].T.astype(np.float32)
    return (h, c), res


def kernel(**inputs):
    (h, c), _ = _run(inputs, BLOC // C)
    return h, c


# revision 49
# speedup vs baseline: 1.0190x; 1.0135x over previous
"""NaryTreeLSTM Trainium2 kernel.

Strategy: pure data-parallel over batch (B=32768 -> 4096 rows/core on 8
cores). All on-device compute happens in transposed [h, batch] layout so
matmuls contract over the SBUF partition dim; activations are pre-cast to
fp16 host-side (child o-gate path additionally to fp8e4 for DoubleRow
matmuls: one 256-deep fp8 matmul costs the same as one 128-deep fp16
matmul). The kernel is ScalarE(ACT)-bound: 24 sigmoid/tanh evaluations
per (b,h) element can only run on the activation-LUT engine (~164us/core
of pure streaming at 1 elem/lane/cycle), so everything else is kept under
that line. Gate preactivations accumulate in PSUM (x-path + hsum-path
summed for free; per-partition biases fused into the ACT instruction; the
node i/o biases are pre-added by a rank-2 indicator matmul so sigmoid(i|o)
is one 1024-wide ACT), k-reductions (hsum, c = i*u + sum_k f_k*cc_k) are
short fp16 DVE tree-adds, outputs ship as fp16 and are upcast host-side.
Emission is software-pipelined per 512-column chunk as child_gates(c) |
node_gates(c-1) | child_tail(c) | node_tail(c-1).
"""

import sys

sys.path.insert(0, "/opt/trn_rl_repo")

import numpy as np

B, K, I, H = 32768, 4, 256, 256
NCORES = 8
BLOC = B // NCORES  # 4096 batch rows per core
C = 512  # chunk columns (one PSUM bank of fp32)

_cache = {}


def _build(nchunk):
    """Build the per-core Bass program (identical on all cores)."""
    import concourse.bass as bass  # noqa: F401
    import bass_rust as _bass_rust
    import concourse.tile as tile
    from concourse import bacc, mybir

    f8 = mybir.dt.float8e4
    f16, f32 = mybir.dt.float16, mybir.dt.float32
    AF = mybir.ActivationFunctionType
    DR = mybir.MatmulPerfMode.DoubleRow

    nc = bacc.Bacc("TRN2", target_bir_lowering=False, debug=False, num_devices=NCORES)

    # DRAM I/O. ax packs, per (chunk, itile): [cx_k0|cx_k1|cx_k2|cx_k3|x]
    # blocks of C columns each, rows = 128 contraction indices.
    ax = nc.dram_tensor("ax", [nchunk, 2, 128, 5 * C], f16, kind="ExternalInput").ap()
    # child data again in fp8 with DoubleRow layout [p, it, col]
    cx8 = nc.dram_tensor("cx8", [nchunk, 128, 2, 4 * C], f8, kind="ExternalInput").ap()
    w8o = nc.dram_tensor("w8o", [128, 2, 256], f8, kind="ExternalInput").ap()
    # per-arity forget weights, fp8 DoubleRow layout [p, ct, k*256+h],
    # scaled x64 (the WfT x-path block is also x64 in fp16; the f-gate
    # sigmoid ACT applies scale=1/64)
    wfk8 = nc.dram_tensor("wfk8", [128, 2, 1024], f8, kind="ExternalInput").ap()
    # wcat cols: 0:768 WxiouT (g*256+h), 768:1024 WfT, 1024:2816 UhT
    # ([Ui|Uo|Uu|WfK0..3] at 1024+blk*256+h); rows = contraction index.
    wcat = nc.dram_tensor("wcat", [2, 128, 2816], f16, kind="ExternalInput").ap()
    # bmat cols: 2g+t for g in {i,o,u,f}, t = h-tile
    bmat = nc.dram_tensor("bmat", [128, 8], f32, kind="ExternalInput").ap()
    # node i/o biases in rank-1-matmul lhsT layout [1, 2*ht+g, h]
    b2mat = nc.dram_tensor("b2mat", [1, 4, 128], f16, kind="ExternalInput").ap()
    # all-ones rhs for the rank-1 bias matmuls
    ind2m = nc.dram_tensor("ind2m", [1, C], f16, kind="ExternalInput").ap()
    h_out = nc.dram_tensor("h_out", [256, nchunk * C], f16, kind="ExternalOutput").ap()
    c_out = nc.dram_tensor("c_out", [256, nchunk * C], f16, kind="ExternalOutput").ap()

    with tile.TileContext(nc) as tc:
        import contextlib

        with contextlib.ExitStack() as ctx:
            wpool = ctx.enter_context(tc.tile_pool(name="w", bufs=1))
            apool = ctx.enter_context(tc.tile_pool(name="a", bufs=3))
            gpool = ctx.enter_context(tc.tile_pool(name="g", bufs=1))
            opool = ctx.enter_context(tc.tile_pool(name="o", bufs=2))
            ppool = ctx.enter_context(tc.tile_pool(name="ps", bufs=2, space="PSUM"))

            # weights on the gpsimd DMA queue so the first ax load (sync
            # queue) runs concurrently; x-path weights first so child
            # matmuls can start before the U-path weights arrive.
            b_sb = wpool.tile([128, 8], f32, tag="bias", name="b_sb")
            nc.gpsimd.dma_start(b_sb[:], bmat[:])
            wA, wB = [], []
            for it in range(2):
                a_ = wpool.tile([128, 1024], f16, tag=f"wA{it}", name=f"wA{it}")
                nc.gpsimd.dma_start(a_[:], wcat[it, :, 0:1024])
                wA.append(a_)
            w8 = wpool.tile([128, 2, 256], f8, tag="w8", name="w8")
            nc.gpsimd.dma_start(w8[:], w8o[:])
            wf8 = wpool.tile([128, 2, 1024], f8, tag="wf8", name="wf8")
            nc.gpsimd.dma_start(wf8[:], wfk8[:])
            b2 = wpool.tile([1, 4, 128], f16, tag="b2", name="b2")
            nc.gpsimd.dma_start(b2[:], b2mat[:])
            for it in range(2):
                b_ = wpool.tile([128, 1792], f16, tag=f"wB{it}", name=f"wB{it}")
                nc.gpsimd.dma_start(b_[:], wcat[it, :, 1024:2816])
                wB.append(b_)
            # all-ones rhs for the rank-1 bias matmuls
            ind2 = wpool.tile([1, C], f16, tag="ind2", name="ind2")
            nc.gpsimd.dma_start(ind2[:], ind2m[:])
            # tiny dummy activations so the ACT table load (~2.7us) happens
            # during the initial DMA wait instead of before the first gate
            warm = wpool.tile([1, 8], f32, tag="warm", name="warm")
            nc.vector.memset(warm[:], 0.0)
            nc.scalar.activation(warm[:], warm[:], AF.Sigmoid)

            def wx(it, col):
                return wA[it][:, col : col + 128]

            def wu(it, col):
                return wB[it][:, col - 1024 : col - 1024 + 128]

            def bias(g, ht):
                # g: 0=i, 1=o, 2=u, 3=f
                return b_sb[:, 2 * g + ht : 2 * g + ht + 1]

            def psum4():
                # uniform 4-bank PSUM tiles; bufs=2 -> all 8 banks in flight
                return ppool.tile([128, 4 * C], f32, tag="p4", name="p4")

            def emit_child(c, split=False):
                """Child (leaf) phase for chunk c. Returns live tiles."""
                a_sb = []
                for it in range(2):
                    a = apool.tile([128, 5 * C], f16, tag=f"a{it}", name=f"a{it}")
                    if split:
                        for k in range(5):
                            nc.sync.dma_start(
                                a[:, k * C : (k + 1) * C],
                                ax[c, it, :, k * C : (k + 1) * C],
                            )
                    else:
                        nc.sync.dma_start(a[:], ax[c, it])
                    a_sb.append(a)
                a8 = apool.tile([128, 2, 4 * C], f8, tag="a8", bufs=2, name="a8")
                nc.sync.dma_start(a8[:], cx8[c])
                gates = {}  # (g, ht) -> [128, 4C] fp16 (4 k-blocks)
                for ht in range(2):
                    for g in range(3):
                        gt = gpool.tile(
                            [128, 4 * C], f16, tag=f"cg{g}{ht}", bufs=2,
                            name=f"cg{g}{ht}",
                        )
                        gates[(g, ht)] = gt
                        col = g * 256 + ht * 128
                        fn = AF.Tanh if g == 2 else AF.Sigmoid
                        ps = psum4()
                        if g == 1:
                            # o-gate: fp8 DoubleRow, 4 matmuls instead of 8
                            for k in range(4):
                                nc.tensor.matmul(
                                    ps[:, k * C : (k + 1) * C],
                                    lhsT=w8[:, :, ht * 128 : ht * 128 + 128],
                                    rhs=a8[:, :, k * C : (k + 1) * C],
                                    start=True,
                                    stop=True,
                                    perf_mode=DR,
                                )
                        else:
                            for it in range(2):  # it-major: same lhsT for 4 MMs
                                for k in range(4):
                                    nc.tensor.matmul(
                                        ps[:, k * C : (k + 1) * C],
                                        lhsT=wx(it, col),
                                        rhs=a_sb[it][:, k * C : (k + 1) * C],
                                        start=(it == 0),
                                        stop=(it == 1),
                                    )
                        nc.scalar.activation(gt[:], ps[:], fn, bias=bias(g, ht))
                return dict(c=c, a=a_sb, gates=gates)

            def emit_child_tail(st, prev2):
                """cc products, merged tanh([cc(c) | c(c-2)]), hsum tree,
                and (two chunks behind) h = o*tanh(c) + its output DMA."""
                c, gates = st["c"], st["gates"]
                tcm, tct, hs = {}, {}, {}
                hs8 = gpool.tile([128, 2, C], f8, tag="hs8", bufs=2, name="hs8")
                for ht in range(2):
                    # tcm = [cc0..cc3 | c(c-2)]; the c slot was written by
                    # node_tail(c-2) last step (absent for chunks 0/1)
                    if st.get("tcm") and ht in st["tcm"]:
                        tcm[ht] = st["tcm"][ht]
                        full = True
                    else:
                        tcm[ht] = gpool.tile(
                            [128, 5 * C], f16, tag=f"tcm{ht}", bufs=3,
                            name=f"tcm{ht}",
                        )
                        full = False
                    nc.vector.tensor_mul(
                        tcm[ht][:, 0 : 4 * C], gates[(0, ht)][:], gates[(2, ht)][:]
                    )
                    tct[ht] = gpool.tile(
                        [128, 5 * C], f16, tag=f"tct{ht}", bufs=1, name=f"tct{ht}"
                    )
                    if full:
                        nc.scalar.activation(tct[ht][:], tcm[ht][:], AF.Tanh)
                    else:
                        nc.scalar.activation(
                            tct[ht][:, 0 : 4 * C], tcm[ht][:, 0 : 4 * C], AF.Tanh
                        )
                    prod = gpool.tile([128, 4 * C], f16, tag=f"prod{ht}", name="prod")
                    nc.vector.tensor_mul(
                        prod[:], gates[(1, ht)][:], tct[ht][:, 0 : 4 * C]
                    )
                    # hsum = sum_k prod_k, via tree adds
                    t2 = gpool.tile([128, 2 * C], f16, tag=f"t2{ht}", name="t2")
                    nc.vector.tensor_add(t2[:], prod[:, 0 : 2 * C], prod[:, 2 * C : 4 * C])
                    hs[ht] = gpool.tile(
                        [128, C], f16, tag=f"hs{ht}", bufs=3, name=f"hs{ht}"
                    )
                    with nc.allow_low_precision("hsum kept in fp16/fp8 for matmul rhs"):
                        nc.vector.tensor_add(hs[ht][:], t2[:, 0:C], t2[:, C : 2 * C])
                        nc.vector.tensor_copy(hs8[:, ht, :], hs[ht][:])
                    if full:
                        p2 = prev2["c"]
                        h_sb = opool.tile([128, C], f16, tag=f"h{ht}", name="h_sb")
                        nc.vector.tensor_mul(
                            h_sb[:], prev2["io_sb"][ht][:, C : 2 * C],
                            tct[ht][:, 4 * C : 5 * C],
                        )
                        rows = slice(ht * 128, (ht + 1) * 128)
                        cols = slice(p2 * C, (p2 + 1) * C)
                        nc.sync.dma_start(h_out[rows, cols], h_sb[:])
                st["tcm"] = tcm
                st["tct"] = tct
                st["hs"] = hs
                st["hs8"] = hs8

            def emit_node(st):
                c, a_sb, hs = st["c"], st["a"], st["hs"]
                xsl = slice(4 * C, 5 * C)

                st["io_sb"], st["u_sb"] = {}, {}
                for ht in range(2):
                    io_sb = gpool.tile(
                        [128, 2 * C], f16, tag=f"io{ht}", bufs=3, name="io_sb"
                    )
                    # i, o, u gates share one 4-bank tile [i|o|u|unused];
                    # i/o biases pre-added via a rank-2 indicator matmul so
                    # sigmoid(i|o) is one 1024-wide ACT
                    ps_iou = psum4()
                    for it in range(2):
                        for g in range(3):
                            nc.tensor.matmul(
                                ps_iou[:, g * C : (g + 1) * C],
                                lhsT=wx(it, g * 256 + ht * 128),
                                rhs=a_sb[it][:, xsl],
                                start=(it == 0),
                                stop=False,
                            )
                    for gb in range(2):
                        nc.tensor.matmul(
                            ps_iou[:, gb * C : (gb + 1) * C],
                            lhsT=b2[:, 2 * ht + gb, :],
                            rhs=ind2[:],
                            start=False,
                            stop=False,
                        )
                    for ct in range(2):
                        for g in range(3):
                            nc.tensor.matmul(
                                ps_iou[:, g * C : (g + 1) * C],
                                lhsT=wu(ct, 1024 + g * 256 + ht * 128),
                                rhs=hs[ct][:],
                                start=False,
                                stop=(ct == 1),
                            )
                    nc.scalar.activation(io_sb[:], ps_iou[:, 0 : 2 * C], AF.Sigmoid)
                    u_sb = gpool.tile([128, C], f16, tag=f"u{ht}", bufs=2, name="u_sb")
                    nc.scalar.activation(
                        u_sb[:], ps_iou[:, 2 * C : 3 * C], AF.Tanh, bias=bias(2, ht)
                    )
                    st["u_sb"][ht] = u_sb
                    # f gates for all 4 children in one 4-bank tile (the
                    # fx = Wf@x term is re-accumulated per k on the PE; the
                    # redundant matmuls are cheaper than any add elsewhere)
                    psf = psum4()
                    for it in range(2):
                        for k in range(4):
                            nc.tensor.matmul(
                                psf[:, k * C : (k + 1) * C],
                                lhsT=wx(it, 768 + ht * 128),
                                rhs=a_sb[it][:, xsl],
                                start=(it == 0),
                                stop=False,
                            )
                    for k in range(4):
                        nc.tensor.matmul(
                            psf[:, k * C : (k + 1) * C],
                            lhsT=wf8[
                                :, :, k * 256 + ht * 128 : k * 256 + ht * 128 + 128
                            ],
                            rhs=st["hs8"][:],
                            start=False,
                            stop=True,
                            perf_mode=DR,
                        )
                    f_sb = gpool.tile([128, 4 * C], f16, tag=f"f{ht}", name="f_sb")
                    nc.scalar.activation(
                        f_sb[:], psf[:], AF.Sigmoid, bias=bias(3, ht), scale=1.0 / 64.0
                    )
                    st["io_sb"][ht] = io_sb
                    st.setdefault("f_sb", {})[ht] = f_sb

            def emit_node_tail(st, pending):
                """c(p) = i*u + sum_k f_k*cc_k, written into the NEXT-NEXT
                chunk's tcm tile (tanh'd there by the merged ACT); c output
                DMA'd now. h(p) happens in child_tail(p+2)."""
                c = st["c"]
                lp = nc.allow_low_precision
                for ht in range(2):
                    f_sb, io_sb = st["f_sb"][ht], st["io_sb"][ht]
                    tcm_p = st["tcm"][ht]
                    prod4 = gpool.tile([128, 4 * C], f16, tag=f"p4d{ht}", name="prod4")
                    nc.vector.tensor_mul(prod4[:], f_sb[:], tcm_p[:, 0 : 4 * C])
                    piu = gpool.tile([128, C], f16, tag=f"piu{ht}", name="piu")
                    nc.vector.tensor_mul(piu[:], io_sb[:, 0:C], st["u_sb"][ht][:])
                    t1 = gpool.tile([128, 2 * C], f16, tag=f"t1{ht}", name="t1")
                    tcm_n = gpool.tile(
                        [128, 5 * C], f16, tag=f"tcm{ht}", bufs=3, name=f"tcm{ht}"
                    )
                    with lp("fp16 tree-add of fp16 products"):
                        nc.vector.tensor_add(
                            t1[:], prod4[:, 0 : 2 * C], prod4[:, 2 * C : 4 * C]
                        )
                        t2 = gpool.tile([128, C], f16, tag=f"t2n{ht}", name="t2n")
                        nc.vector.tensor_add(t2[:], t1[:, 0:C], t1[:, C : 2 * C])
                        nc.vector.tensor_add(
                            tcm_n[:, 4 * C : 5 * C], t2[:], piu[:]
                        )
                    rows = slice(ht * 128, (ht + 1) * 128)
                    cols = slice(c * C, (c + 1) * C)
                    nc.sync.dma_start(c_out[rows, cols], tcm_n[:, 4 * C : 5 * C])
                    pending[ht] = tcm_n

            # Software pipeline per step c:
            #   child_gates(c) | node_gates(c-1) | child_tail(c) (merged
            #   tanh + h(c-2)) | node_tail(c-1) (c into tcm(c+1))
            prev, prev2 = None, None
            pending = {}
            for c in range(nchunk):
                cur = emit_child(c, split=(c == 0))
                if pending:
                    cur["tcm"] = dict(pending)
                    pending.clear()
                # last chunk: child_tail first so hs(last) is ready early
                # and the epilogue's node fills overlap this step's ACTs
                if c == nchunk - 1:
                    emit_child_tail(cur, prev2)
                    if prev is not None:
                        emit_node(prev)
                else:
                    if prev is not None:
                        emit_node(prev)
                    emit_child_tail(cur, prev2)
                if prev is not None:
                    emit_node_tail(prev, pending)
                prev2, prev = prev, cur
            emit_node(prev)
            emit_node_tail(prev, pending2 := {})
            # epilogue: tanh + h for the last two chunks (their c landed in
            # tcm tiles that no later child_tail will tanh)
            for st, pend in ((prev2, pending), (prev, pending2)):
                p = st["c"]
                for ht in range(2):
                    tcn = pend[ht]
                    tc_sb = gpool.tile([128, C], f16, tag=f"tc{ht}", name="tc_sb")
                    nc.scalar.activation(
                        tc_sb[:], tcn[:, 4 * C : 5 * C], AF.Tanh
                    )
                    h_sb = opool.tile([128, C], f16, tag=f"h{ht}", name="h_sb")
                    nc.vector.tensor_mul(
                        h_sb[:], st["io_sb"][ht][:, C : 2 * C], tc_sb[:]
                    )
                    rows = slice(ht * 128, (ht + 1) * 128)
                    cols = slice(p * C, (p + 1) * C)
                    nc.sync.dma_start(h_out[rows, cols], h_sb[:])

    nc.compile()
    return nc


def _prep_shared(Wi, bi, Wf, bf, Wo, bo, Wu, bu, Ui, Uo, Uu, WfK):
    """Weight/bias packing shared by all cores."""
    import ml_dtypes

    WxiouT = np.concatenate([Wi, Wo, Wu], axis=0).T  # [256, 768]
    WfT = 64.0 * np.asarray(Wf, np.float32).T  # [256, 256], x64 for fp8 WfK path
    UhT = np.concatenate([Ui, Uo, Uu, WfK[0], WfK[1], WfK[2], WfK[3]], axis=0).T
    wcat = np.concatenate([WxiouT, WfT, UhT], axis=1).astype(np.float16)  # [256, 2816]
    wcat = np.ascontiguousarray(wcat.reshape(2, 128, 2816))

    bmat = np.empty((128, 8), np.float32)
    for g, b in enumerate([bi, bo, bu, bf]):
        b = np.asarray(b, np.float32)
        bmat[:, 2 * g] = b[:128]
        bmat[:, 2 * g + 1] = b[128:]

    b2mat = np.empty((1, 4, 128), np.float16)
    for ht in range(2):
        b2mat[0, 2 * ht + 0] = np.asarray(bi, np.float32)[ht * 128 : (ht + 1) * 128]
        b2mat[0, 2 * ht + 1] = np.asarray(bo, np.float32)[ht * 128 : (ht + 1) * 128]

    ind2m = np.ones((1, C), np.float16)

    # fp8 o-gate weights, DoubleRow layout [p, it, hcol]
    w8o = np.ascontiguousarray(
        np.asarray(Wo, np.float32).T.reshape(2, 128, 256).transpose(1, 0, 2)
    ).astype(ml_dtypes.float8_e4m3)
    # fp8 forget weights x64, DoubleRow layout [p, ct, k*256+h]
    wfkT = np.concatenate(
        [64.0 * np.asarray(WfK[k], np.float32).T for k in range(4)], axis=1
    )  # [256, 1024]
    wfk8 = np.ascontiguousarray(
        wfkT.reshape(2, 128, 1024).transpose(1, 0, 2)
    ).astype(ml_dtypes.float8_e4m3)
    return wcat, bmat, b2mat, ind2m, w8o, wfk8


def _prep_core(x, child_x, m, nchunk):
    """Pack per-core activations: [nchunk, 2, 128, 5C] fp16 + fp8 child."""
    import ml_dtypes

    bloc = nchunk * C
    sl = slice(m * bloc, (m + 1) * bloc)
    cxt = np.asarray(child_x[sl], np.float16).transpose(2, 1, 0)  # [256, 4, bloc]
    xt = np.asarray(x[sl], np.float16).T[:, None, :]  # [256, 1, bloc]
    full = np.concatenate([cxt, xt], axis=1)  # [256, 5, bloc]
    # [it, p, j, chunk, cb] -> [chunk, it, p, j, cb]
    full = full.reshape(2, 128, 5, nchunk, C).transpose(3, 0, 1, 2, 4)
    ax = np.ascontiguousarray(full).reshape(nchunk, 2, 128, 5 * C)
    # fp8 child blocks only, DoubleRow layout [chunk, p, it, 4C]
    cx8 = np.ascontiguousarray(
        ax[:, :, :, 0 : 4 * C].transpose(0, 2, 1, 3)
    ).astype(ml_dtypes.float8_e4m3)
    return ax, cx8


def _run(inputs, nchunk, trace=False):
    from concourse.bass_utils import run_bass_kernel_spmd

    key = ("nc", nchunk)
    if key not in _cache:
        _cache[key] = _build(nchunk)
    nc = _cache[key]

    wcat, bmat, b2mat, ind2m, w8o, wfk8 = _prep_shared(
        inputs["Wi"], inputs["bi"], inputs["Wf"], inputs["bf"],
        inputs["Wo"], inputs["bo"], inputs["Wu"], inputs["bu"],
        inputs["Ui"], inputs["Uo"], inputs["Uu"], inputs["WfK"],
    )
    in_maps = []
    for m in range(NCORES):
        ax, cx8 = _prep_core(inputs["x"], inputs["child_x"], m, nchunk)
        in_maps.append(
            {"ax": ax, "cx8": cx8, "wcat": wcat, "bmat": bmat,
             "b2mat": b2mat, "ind2m": ind2m, "w8o": w8o, "wfk8": wfk8}
        )

    res = run_bass_kernel_spmd(
        nc, in_maps, core_ids=list(range(NCORES)), trace=trace
    )
    bloc = nchunk * C
    h = np.empty((NCORES * bloc, 256), np.float32)
    c = np.empty((NCORES * bloc, 256), np.float32)
    for m, r in enumerate(res.results):
        h[m * bloc : (m + 1) * bloc] = r["h_out"].T.astype(np.float32)
        c[m * bloc : (m + 1) * bloc] = r["c_out"].T.astype(np.float32)
    return (h, c), res


def kernel(**inputs):
    (h, c), _ = _run(inputs, BLOC // C)
    return h, c
